# revision 13
# baseline (speedup 1.0000x reference)
"""GAU (Gated Attention Unit) fused kernel for Trainium2, SPMD over 8 NeuronCores.

Sharding: data-parallel over batch (B=4) x query-sequence-halves (2) = 8 cores.
Each core computes the full GAU for its (batch, query-half): LN -> qk/v/gate
projections -> relu^2 attention with T5 relative bias -> gated output
projection -> residual.  No cross-core communication; k/v are recomputed per
query-half (cheap relative to attention).

Layout strategy (all matmuls bf16 inputs, fp32 PSUM accumulation):
  - normed^T [d-part, s] produced via PE transpose; used as lhsT for v-proj
    and as rhs for k/q/gate projections.
  - k^T/q^T [qk-dim-part, s]; sim^T[j,i] tiles via one matmul (K=qk=128).
  - attn2 = relu(sim + bias)^2 computed as (s1 max 0)*s1 in one DVE op.
  - out[i,h] accumulated with lhsT=attn2 slices, rhs=v row-tiles streamed
    contiguously from DRAM scratch.
  - gate multiply in natural [i,h]; PE-transpose the gated tiles to feed the
    final projection (lhsT=[h,i], rhs=Wo).
  - T5 rel-bias folded to a host-precomputed per-core table BT[128, S-128+SQ],
    so in-kernel bias-add is a plain SBUF slice add (1/seq_len folded in).
"""

import math
import os
import sys

for _p in ("/opt/trn_rl_repo", "/root/.axon_site/_ro/trn_rl_repo"):
    if os.path.isdir(_p) and _p not in sys.path:
        sys.path.append(_p)

import numpy as np
import ml_dtypes

import concourse.bass as bass
import concourse.tile as tile
from concourse import mybir
from concourse.bass_utils import run_bass_kernel_spmd
from concourse.masks import make_identity

# Problem dims (hardcoded per spec)
B, S, D, QK, H = 4, 4096, 1024, 128, 2048
NUM_BUCKETS, MAX_DIST = 32, 128
LN_EPS = 1e-5
N_CORES = 8

P = 128
NB = 512  # free-dim block for matmuls

BF16 = mybir.dt.bfloat16
F32 = mybir.dt.float32

_NC_CACHE = {}


def _split_excess_waits(nc, max_waits=1):
    """This container's walrus rejects instructions carrying more than one
    sem wait ("Too many sync wait commands").  Move excess waits onto
    same-engine nops inserted immediately before the instruction — engine
    FIFO order makes that semantically identical."""
    f = nc.m.functions[0]
    for bb in list(f.blocks):
        il = list(bb.instructions)
        out = []
        changed = False
        for inst in il:
            si = inst.sync_info
            if si is not None and si.on_wait and len(si.on_wait) > max_waits:
                waits = list(si.on_wait)
                moved, keep = waits[:-max_waits], waits[-max_waits:]
                si.on_wait = keep
                for w in moved:
                    eng = nc.engines[inst.engine]
                    cur_bb = nc.cur_bb.bb
                    n_before = len(cur_bb.instructions)
                    nop = eng.nop()
                    # pop the freshly appended nop from wherever it landed
                    tail = list(cur_bb.instructions)
                    assert tail[-1] is nop.ins and len(tail) == n_before + 1
                    cur_bb.instructions = tail[:-1]
                    nsi = nop.ins.sync_info
                    if nsi is None:
                        nop.ins.sync_info = mybir.SyncInfo(
                            on_wait=[w], on_update=[])
                    else:
                        nsi.on_wait = [w]
                    out.append(nop.ins)
                changed = True
            out.append(inst)
        if changed:
            bb.instructions = out


def _install_drain_wait_split():
    """The walrus build in this container rejects >1 sem wait on the Tile
    epilogue Drain ("Too many sync wait commands").  Split the extra waits
    onto explicit SP nops (they only need to precede the final barrier)."""
    from concourse.vector_clock import ScopedClock

    if getattr(tile.TileContext, "_drain_split_installed", False):
        return

    def _patched(self, tick_clock, wait_clock):
        drain_inst = self.nc.sync.drain()
        wait_clock.add_sem_waits(
            drain_inst.ins, ScopedClock({None: tick_clock.global_clock}))
        si = drain_inst.ins.sync_info
        if si is not None and si.on_wait and len(si.on_wait) > 1:
            extra = list(si.on_wait)[1:]
            si.on_wait = [si.on_wait[0]]
            for w in extra:
                nop = self.nc.sync.nop()
                nsi = nop.ins.sync_info
                if nsi is None:
                    nop.ins.sync_info = mybir.SyncInfo(on_wait=[w], on_update=[])
                else:
                    nsi.on_wait = [w]
        self.nc.all_engine_barrier()
        assert self.sems is not None
        popped = self.nc._tile_sem_poison_stack.pop()
        assert popped is self._sem_poison
        self.nc.clear_and_free_semaphores(list(self.sems.allocated().values()))
        self.nc.all_engine_barrier()

    tile.TileContext._drain_and_barrier = _patched
    tile.TileContext._drain_split_installed = True


_install_drain_wait_split()


def build_gau_nc(S=S, SQ=S // 2, D=D, QK=QK, H=H):
    """Build the SPMD Bass program for one core: full-seq keys/values,
    SQ query rows."""
    assert D % P == 0 and H % P == 0 and S % NB == 0 and SQ % NB == 0
    assert QK == P
    KD = D // P      # d chunks
    NSK = S // P     # key-side seq tiles
    NSQ = SQ // P    # query-side seq tiles
    SBK = S // NB    # key-side 512-blocks
    IB = SQ // NB    # query-side 512-blocks (i blocks)
    HC = H // P      # h 128-chunks
    HB = H // NB     # h 512-blocks
    JC = S // P      # j chunks
    DB = D // NB     # output d blocks
    ISUB = NB // P   # i subtiles per i-block
    W = S - P + SQ   # bias table width
    HQ = H // NB     # h quarters (512 each)
    D_HALF = min(512, D)  # bn_stats max free dim

    nc = bass.Bass("TRN2", target_bir_lowering=False, debug=False)

    # ---- DRAM I/O ----
    xk = nc.dram_tensor("xk", [S, D], F32, kind="ExternalInput").ap()
    xq = nc.dram_tensor("xq", [SQ, D], F32, kind="ExternalInput").ap()
    whv = nc.dram_tensor("whv", [D, H], BF16, kind="ExternalInput").ap()
    whg = nc.dram_tensor("whg", [D, H], BF16, kind="ExternalInput").ap()
    wqk = nc.dram_tensor("wqk", [D, QK], BF16, kind="ExternalInput").ap()
    wo = nc.dram_tensor("wo", [H, D], BF16, kind="ExternalInput").ap()
    lng = nc.dram_tensor("lng", [D], F32, kind="ExternalInput").ap()
    lnb = nc.dram_tensor("lnb", [D], F32, kind="ExternalInput").ap()
    bqk = nc.dram_tensor("bqk", [QK], F32, kind="ExternalInput").ap()
    g0 = nc.dram_tensor("g0", [QK], F32, kind="ExternalInput").ap()  # gamma0/S
    b0 = nc.dram_tensor("b0", [QK], F32, kind="ExternalInput").ap()  # beta0/S
    g1 = nc.dram_tensor("g1", [QK], F32, kind="ExternalInput").ap()
    b1 = nc.dram_tensor("b1", [QK], F32, kind="ExternalInput").ap()
    bhv = nc.dram_tensor("bhv", [H], F32, kind="ExternalInput").ap()
    bhg = nc.dram_tensor("bhg", [H], F32, kind="ExternalInput").ap()
    bo = nc.dram_tensor("bo", [D], F32, kind="ExternalInput").ap()
    bt = nc.dram_tensor("bt", [P, W], BF16, kind="ExternalInput").ap()
    out = nc.dram_tensor("out", [SQ, D], F32, kind="ExternalOutput").ap()

    # DRAM scratch
    vsc = nc.dram_tensor("vsc", [S, H], BF16, kind="Internal").ap()
    gsc = nc.dram_tensor("gsc", [SQ, H], BF16, kind="Internal").ap()

    with tile.TileContext(nc) as tc:
        from contextlib import ExitStack

        with ExitStack() as outer:
            # pools that live for the whole kernel
            singles = outer.enter_context(tc.tile_pool(name="singles", bufs=1))
            wpool = outer.enter_context(tc.tile_pool(name="wpool", bufs=1))
            qkpool = outer.enter_context(tc.tile_pool(name="qkpool", bufs=1))
            ps_small = outer.enter_context(
                tc.tile_pool(name="ps_small", bufs=2, space="PSUM"))
            ps_mm = outer.enter_context(
                tc.tile_pool(name="ps_mm", bufs=2, space="PSUM"))

            # identity for PE transpose (bf16)
            ident = singles.tile([P, P], BF16)
            make_identity(nc, ident)

            eps_sb = singles.tile([P, 1], F32)
            nc.vector.memset(eps_sb, LN_EPS)

            # small parameter tiles
            lng_sb = singles.tile([P, KD], F32)
            nc.sync.dma_start(lng_sb, lng.rearrange("(o p) -> p o", p=P))
            lnb_sb = singles.tile([P, KD], F32)
            nc.sync.dma_start(lnb_sb, lnb.rearrange("(o p) -> p o", p=P))
            bqk_sb = singles.tile([P, 1], F32)
            nc.sync.dma_start(bqk_sb, bqk.unsqueeze(1))
            g0_sb = singles.tile([P, 1], F32)
            nc.sync.dma_start(g0_sb, g0.unsqueeze(1))
            b0_sb = singles.tile([P, 1], F32)
            nc.sync.dma_start(b0_sb, b0.unsqueeze(1))
            g1_sb = singles.tile([P, 1], F32)
            nc.sync.dma_start(g1_sb, g1.unsqueeze(1))
            b1_sb = singles.tile([P, 1], F32)
            nc.sync.dma_start(b1_sb, b1.unsqueeze(1))
            bhv_sb = singles.tile([P, H], F32)
            nc.sync.dma_start(bhv_sb, bhv.unsqueeze(0).to_broadcast((P, H)))
            bhg_sb = singles.tile([P, H], F32)
            nc.sync.dma_start(bhg_sb, bhg.unsqueeze(0).to_broadcast((P, H)))
            bo_sb = singles.tile([P, D], F32)
            nc.sync.dma_start(bo_sb, bo.unsqueeze(0).to_broadcast((P, D)))
            bt_sb = qkpool.tile([P, W], BF16)
            nc.sync.dma_start(bt_sb, bt)

            wqk_sb = qkpool.tile([P, KD, QK], BF16)
            nc.sync.dma_start(wqk_sb, wqk.rearrange("(o p) q -> p o q", p=P))

            kT = qkpool.tile([P, S], BF16)   # [qk-dim, s]
            qT = qkpool.tile([P, SQ], BF16)  # [qk-dim, i]

            with ExitStack() as ph12:
                big = ph12.enter_context(tc.tile_pool(name="big", bufs=1))
                work = ph12.enter_context(tc.tile_pool(name="work", bufs=3))
                stat = ph12.enter_context(tc.tile_pool(name="stat", bufs=4))
                rowp = ph12.enter_context(tc.tile_pool(name="rowp", bufs=2))
                ps_tr = ph12.enter_context(
                    tc.tile_pool(name="ps_tr", bufs=4, space="PSUM"))

                ntk = big.tile([P, KD, S], BF16, tag="ntk")    # normed^T keys
                ntq = big.tile([P, KD, SQ], BF16, tag="ntq")   # normed^T queries

                # ---------- Phase 0: LayerNorm + transpose ----------
                def layernorm_into(x_ap, n_tiles, ntT):
                    nsub = D // D_HALF
                    for t in range(n_tiles):
                        x_t = work.tile([P, D], F32, tag="xt")
                        nc.sync.dma_start(x_t, x_ap[t * P:(t + 1) * P, :])
                        stats = stat.tile([P, nsub, 6], F32, tag="st")
                        for i in range(nsub):
                            nc.vector.bn_stats(
                                out=stats[:, i, :],
                                in_=x_t[:, i * D_HALF:(i + 1) * D_HALF])
                        mv = stat.tile([P, 2], F32, tag="mv")
                        nc.vector.bn_aggr(out=mv, in_=stats)
                        rstd = stat.tile([P, 1], F32, tag="rs")
                        nc.scalar.activation(
                            out=rstd, in_=mv[:, 1:2],
                            func=mybir.ActivationFunctionType.Sqrt,
                            bias=eps_sb, scale=1.0)
                        nc.vector.reciprocal(out=rstd, in_=rstd)
                        nm = stat.tile([P, 1], F32, tag="nm")
                        nc.vector.tensor_mul(nm, mv[:, 0:1], rstd)
                        nc.scalar.mul(nm, nm, -1.0)
                        nrm = work.tile([P, D], BF16, tag="nrm")
                        nc.scalar.activation(
                            out=nrm, in_=x_t,
                            func=mybir.ActivationFunctionType.Identity,
                            bias=nm, scale=rstd)
                        for k in range(KD):
                            pst = ps_tr.tile([P, P], BF16, tag="pst")
                            nc.tensor.transpose(
                                pst, nrm[:, k * P:(k + 1) * P], ident)
                            nc.scalar.activation(
                                out=ntT[:, k, t * P:(t + 1) * P], in_=pst,
                                func=mybir.ActivationFunctionType.Identity,
                                bias=lnb_sb[:, k:k + 1],
                                scale=lng_sb[:, k:k + 1])

                layernorm_into(xk, NSK, ntk)
                layernorm_into(xq, NSQ, ntq)

                # ---------- Phase 1: k^T / q^T projections ----------
                def qk_proj(ntT, nblk, dstT, gg, bb):
                    for sb in range(nblk):
                        ps = ps_mm.tile([P, NB], F32, tag="mm")
                        for k in range(KD):
                            nc.tensor.matmul(
                                ps, wqk_sb[:, k, :],
                                ntT[:, k, sb * NB:(sb + 1) * NB],
                                start=(k == 0), stop=(k == KD - 1))
                        tmp = work.tile([P, NB], F32, tag="qtmp")
                        nc.scalar.activation(
                            out=tmp, in_=ps,
                            func=mybir.ActivationFunctionType.Silu,
                            bias=bqk_sb, scale=1.0)
                        nc.vector.tensor_scalar(
                            out=dstT[:, sb * NB:(sb + 1) * NB],
                            in0=tmp, scalar1=gg, scalar2=bb,
                            op0=mybir.AluOpType.mult,
                            op1=mybir.AluOpType.add)

                qk_proj(ntk, SBK, kT, g1_sb, b1_sb)
                qk_proj(ntq, IB, qT, g0_sb, b0_sb)

                # ---------- Phase 2a: v projection -> DRAM ----------
                whv_sb = wpool.tile([P, KD, H], BF16, tag="w")
                nc.sync.dma_start(whv_sb, whv.rearrange("(o p) h -> p o h", p=P))
                for st in range(NSK):
                    vrow = rowp.tile([P, H], BF16, tag="vrow")
                    for hb in range(HB):
                        ps = ps_mm.tile([P, NB], F32, tag="mm")
                        for k in range(KD):
                            nc.tensor.matmul(
                                ps, ntk[:, k, st * P:(st + 1) * P],
                                whv_sb[:, k, hb * NB:(hb + 1) * NB],
                                start=(k == 0), stop=(k == KD - 1))
                        nc.vector.tensor_add(
                            out=ps, in0=ps,
                            in1=bhv_sb[:, hb * NB:(hb + 1) * NB])
                        nc.scalar.activation(
                            out=vrow[:, hb * NB:(hb + 1) * NB], in_=ps,
                            func=mybir.ActivationFunctionType.Silu)
                    nc.sync.dma_start(vsc[st * P:(st + 1) * P, :], vrow)

                # ---------- Phase 2b: gate projection -> DRAM ----------
                whg_sb = wpool.tile([P, KD, H], BF16, tag="w")
                nc.sync.dma_start(whg_sb, whg.rearrange("(o p) h -> p o h", p=P))
                for st in range(NSQ):
                    grow = rowp.tile([P, H], BF16, tag="vrow")
                    for hb in range(HB):
                        ps = ps_mm.tile([P, NB], F32, tag="mm")
                        for k in range(KD):
                            nc.tensor.matmul(
                                ps, ntq[:, k, st * P:(st + 1) * P],
                                whg_sb[:, k, hb * NB:(hb + 1) * NB],
                                start=(k == 0), stop=(k == KD - 1))
                        nc.vector.tensor_add(
                            out=ps, in0=ps,
                            in1=bhg_sb[:, hb * NB:(hb + 1) * NB])
                        nc.scalar.activation(
                            out=grow[:, hb * NB:(hb + 1) * NB], in_=ps,
                            func=mybir.ActivationFunctionType.Silu)
                    nc.sync.dma_start(gsc[st * P:(st + 1) * P, :], grow)

            # ---------- Phase 3: attention + gating + out-proj ----------
            wo_sb = wpool.tile([P, HC, D], BF16, tag="w")
            nc.sync.dma_start(wo_sb, wo.rearrange("(o p) d -> p o d", p=P))

            with ExitStack() as ph3:
                apool = ph3.enter_context(tc.tile_pool(name="apool", bufs=1))
                vpool = ph3.enter_context(tc.tile_pool(name="vpool", bufs=3))
                gpool = ph3.enter_context(tc.tile_pool(name="gpool", bufs=4))
                opool = ph3.enter_context(tc.tile_pool(name="opool", bufs=3))
                ps_acc = ph3.enter_context(
                    tc.tile_pool(name="ps_acc", bufs=ISUB, space="PSUM"))

                VG = 4  # j-chunks per v DMA
                for ib in range(IB):
                    # sim + relu^2 for the whole i-block
                    attn2 = apool.tile([P, JC, NB], BF16, tag="attn2")
                    for j in range(JC):
                        ps = ps_small.tile([P, NB], F32, tag="pssim")
                        nc.tensor.matmul(
                            ps, kT[:, j * P:(j + 1) * P],
                            qT[:, ib * NB:(ib + 1) * NB],
                            start=True, stop=True)
                        m0 = ib * NB - j * P + (S - P)
                        s1 = opool.tile([P, NB], F32, tag="s1")
                        nc.vector.tensor_add(
                            out=s1, in0=ps, in1=bt_sb[:, m0:m0 + NB])
                        nc.vector.scalar_tensor_tensor(
                            out=attn2[:, j, :], in0=s1, scalar=0.0, in1=s1,
                            op0=mybir.AluOpType.max,
                            op1=mybir.AluOpType.mult)

                    # attn2 @ v, h in 512-wide quarters; gate; transpose
                    goT = apool.tile([P, HC, NB], BF16, tag="goT")
                    for hq in range(HQ):
                        pacc = [ps_acc.tile([P, NB], F32, tag="pacc",
                                            name=f"pacc{_i}")
                                for _i in range(ISUB)]
                        for jg in range(JC // VG):
                            vt = vpool.tile([P, VG, NB], BF16, tag="vt")
                            nc.sync.dma_start(
                                vt,
                                vsc.rearrange("(o p) h -> p o h", p=P)
                                [:, jg * VG:(jg + 1) * VG,
                                 hq * NB:(hq + 1) * NB])
                            for jj in range(VG):
                                j = jg * VG + jj
                                for isub in range(ISUB):
                                    nc.tensor.matmul(
                                        pacc[isub],
                                        attn2[:, j, isub * P:(isub + 1) * P],
                                        vt[:, jj, :],
                                        start=(j == 0), stop=(j == JC - 1))
                        # gate-multiply + transpose into goT
                        for isub in range(ISUB):
                            i0 = ib * NB + isub * P
                            gt = gpool.tile([P, NB], BF16, tag="gt")
                            nc.sync.dma_start(
                                gt, gsc[i0:i0 + P, hq * NB:(hq + 1) * NB])
                            gated = gpool.tile([P, NB], BF16, tag="gated")
                            nc.vector.tensor_mul(gated, pacc[isub], gt)
                            for hh in range(NB // P):
                                hc = hq * (NB // P) + hh
                                pst = ps_mm.tile([P, P], BF16, tag="mm")
                                nc.tensor.transpose(
                                    pst, gated[:, hh * P:(hh + 1) * P], ident)
                                nc.scalar.copy(
                                    out=goT[:, hc, isub * P:(isub + 1) * P],
                                    in_=pst)

                    # out projection + bias + residual
                    for isub in range(ISUB):
                        i0 = ib * NB + isub * P
                        for db in range(DB):
                            ps = ps_mm.tile([P, NB], F32, tag="mm")
                            for hc in range(HC):
                                nc.tensor.matmul(
                                    ps, goT[:, hc, isub * P:(isub + 1) * P],
                                    wo_sb[:, hc, db * NB:(db + 1) * NB],
                                    start=(hc == 0), stop=(hc == HC - 1))
                            xt = opool.tile([P, NB], F32, tag="xres")
                            nc.sync.dma_start(
                                xt, xq[i0:i0 + P, db * NB:(db + 1) * NB])
                            nc.vector.tensor_add(
                                out=ps, in0=ps,
                                in1=bo_sb[:, db * NB:(db + 1) * NB])
                            ot = opool.tile([P, NB], F32, tag="ot")
                            nc.vector.tensor_add(out=ot, in0=ps, in1=xt)
                            nc.sync.dma_start(
                                out[i0:i0 + P, db * NB:(db + 1) * NB], ot)

    _split_excess_waits(nc)
    return nc


def _t5_bias_vec(rel_emb, S_, D_):
    """bv[r + S_-1] = bias for rel = k_pos - q_pos = r, scaled by sqrt(D)/S."""
    r = np.arange(-(S_ - 1), S_, dtype=np.int64)
    n = (-r).astype(np.int64)
    nb = NUM_BUCKETS // 2
    me = nb // 2
    ret = (n < 0).astype(np.int64) * nb
    na = np.abs(n)
    val_large = me + (
        np.log(np.maximum(na, 1).astype(np.float32) / me)
        / math.log(MAX_DIST / me) * (nb - me)).astype(np.int64)
    val_large = np.minimum(val_large, nb - 1)
    bucket = ret + np.where(na < me, na, val_large)
    return (rel_emb[bucket, 0].astype(np.float64)
            * (float(D_) ** 0.5) / float(S_)).astype(np.float32)


def make_core_inputs(inputs, S_=S, SQ_=None, D_=D, QK_=QK, H_=H,
                     n_cores=N_CORES):
    """Build per-core in_maps from the full (unsharded) input dict."""
    if SQ_ is None:
        SQ_ = S_ // 2
    bf = ml_dtypes.bfloat16
    x = np.asarray(inputs["x"], np.float32)
    Wh = np.asarray(inputs["Wh"], np.float32)
    bh = np.asarray(inputs["bh"], np.float32)
    Wqk = np.asarray(inputs["Wqk"], np.float32)
    bqk_ = np.asarray(inputs["bqk"], np.float32)
    osg = np.asarray(inputs["os_gamma"], np.float32)
    osb = np.asarray(inputs["os_beta"], np.float32)
    Wo = np.asarray(inputs["Wo"], np.float32)
    bo_ = np.asarray(inputs["bo"], np.float32)
    rel_emb = np.asarray(inputs["rel_emb"], np.float32)
    lng_ = np.asarray(inputs["ln_g"], np.float32)
    lnb_ = np.asarray(inputs["ln_b"], np.float32)

    bv = _t5_bias_vec(rel_emb, S_, D_)
    W_ = S_ - P + SQ_
    halves = S_ // SQ_

    shared = dict(
        whv=np.ascontiguousarray(Wh[:, :H_]).astype(bf),
        whg=np.ascontiguousarray(Wh[:, H_:]).astype(bf),
        wqk=np.ascontiguousarray(Wqk).astype(bf),
        wo=np.ascontiguousarray(Wo).astype(bf),
        lng=lng_, lnb=lnb_,
        bqk=bqk_,
        g0=(osg[0] / float(S_)).astype(np.float32),
        b0=(osb[0] / float(S_)).astype(np.float32),
        g1=osg[1].copy(), b1=osb[1].copy(),
        bhv=np.ascontiguousarray(bh[:H_]),
        bhg=np.ascontiguousarray(bh[H_:]),
        bo=bo_,
    )

    pp = np.arange(P)[:, None]
    mm = np.arange(W_)[None, :]
    in_maps = []
    for c in range(n_cores):
        b = c // halves
        off = (c % halves) * SQ_
        idx = pp - mm + (S_ - P) - off + (S_ - 1)
        btc = bv[idx].astype(bf)
        m = dict(shared)
        m["xk"] = np.ascontiguousarray(x[b])
        m["xq"] = np.ascontiguousarray(x[b, off:off + SQ_])
        m["bt"] = btc
        in_maps.append(m)
    return in_maps


def run_with_results(inputs, trace=False):
    key = (S, S // 2, D, QK, H)
    if key not in _NC_CACHE:
        _NC_CACHE[key] = build_gau_nc(*key)
    nc = _NC_CACHE[key]
    in_maps = make_core_inputs(inputs)
    res = run_bass_kernel_spmd(nc, in_maps, core_ids=list(range(N_CORES)),
                               trace=trace)
    SQ_ = S // 2
    halves = S // SQ_
    out = np.empty((B, S, D), np.float32)
    for c in range(N_CORES):
        b = c // halves
        off = (c % halves) * SQ_
        out[b, off:off + SQ_, :] = res.results[c]["out"]
    return out, res


def kernel(**inputs):
    return run_with_results(inputs, trace=False)[0]
